# revision 30
# baseline (speedup 1.0000x reference)
"""Trainium2 Bass kernel for nn_Decoder_26585847562664 (v3).

16-head causal attention decoder: B=2, S=2048, D=1024, HD=64.
Sharded over 8 NeuronCores as (batch x head-group): core c handles batch
c//4 and heads [4*(c%4), 4*(c%4)+4) -- Wq/Wk/Wv are split column-wise by
head group on the host, so every core's work is fully local (no
collectives).

v3 changes vs v2:
- Scores for a head PAIR are computed by two row-tiled concurrent K=64
  matmuls (partitions 0-63 / 64-127) into one [128, 2, 512] psum tile;
  ONE exp activation covers both heads (80 activations, fewer+wider).
- Causal mask applied additively PRE-exp by an identity-weight matmul
  adding -30000*(1-mask) into score psum; all DVE mask multiplies gone.
- bv folded into v_sb rows (numerator absorbs Z*bv), so normalization
  needs no bias add; reciprocals batched [128, 4] per head.
- Q/K bias psum->sbuf moves on DVE (tensor_scalar add); the Scalar
  engine runs exp only.
- V activations arrive in fp8 (halves the largest input DMA); V
  projection is fp8 stationary x bf16 moving.
- Projection chains drip through attention on a deadline schedule to
  keep the PE HAM-warm.
"""

import json
import os
import sys
import types

import numpy as np
import ml_dtypes

B, S, D, H = 2, 2048, 1024, 16
HD = 64
NH = 4            # heads per core
OC = NH * HD      # 256 projection columns per core
NB = S // 128     # 16 row blocks
QC = S // 512     # 4 q-chunks of 512
DCH = D // 128    # 8 contraction chunks (bf16 path)
DC2 = D // 256    # 4 fused contraction chunks (fp8 DoubleRow path)
SCALE = 1.0 / 32.0  # 1/sqrt(D)
MASK_NEG = -30000.0

BF16 = ml_dtypes.bfloat16
F8 = ml_dtypes.float8_e4m3

_cache = {}


# --------------------------------------------------------------------------
# environment shims (walrus single-wait limit, missing NTFF hook, no egress)
# --------------------------------------------------------------------------

def _install_shims():
    import concourse.bass as bass

    if not getattr(bass.Bass.to_json_bytes, "_wait_split", False):
        orig = bass.Bass.to_json_bytes

        def to_json_bytes(self):
            m = json.loads(orig(self))
            for fn in m.get("functions", []):
                for bb in fn.get("blocks", []):
                    out = []
                    for inst in bb.get("instructions", []):
                        si = inst.get("sync_info")
                        waits = (si or {}).get("on_wait") or []
                        if len(waits) > 1:
                            for j, w in enumerate(waits[:-1]):
                                out.append({
                                    "debug": inst.get("debug", 0),
                                    "engine": inst["engine"],
                                    "ins": [],
                                    "name": f"{inst['name']}ws{j}",
                                    "opcode": "NoOp",
                                    "outs": [],
                                    "sync_info": {"on_update": [], "on_wait": [w]},
                                })
                            si["on_wait"] = [waits[-1]]
                        out.append(inst)
                    bb["instructions"] = out
            return json.dumps(m).encode()

        to_json_bytes._wait_split = True
        bass.Bass.to_json_bytes = to_json_bytes

    try:
        import antenv
        try:
            from antenv import axon_hooks  # noqa: F401
        except ImportError:
            from trn_agent_boot.trn_boot import _ntff_profile_via_ctypes

            mod = types.ModuleType("antenv.axon_hooks")
            hook = [_ntff_profile_via_ctypes("/opt/axon/libaxon_pjrt.so")]
            mod.get_axon_ntff_profile_hook = lambda: hook[0]
            mod.set_axon_ntff_profile_hook = lambda h: hook.__setitem__(0, h)
            sys.modules["antenv.axon_hooks"] = mod
            antenv.axon_hooks = mod
    except Exception:
        pass

    try:
        from concourse import bass_utils
        bass_utils.upload_artifacts = lambda tmpdir: "local://skipped"
    except Exception:
        pass


# --------------------------------------------------------------------------
# mask block classification (host side)
# --------------------------------------------------------------------------

def _classify_mask(m2):
    """m2: [S, S] int array, m2[q, k] == 1 -> position attended.

    Returns (kind, mtile_idx, mtiles):
      kind[kb][qb]  in {0 zero, 1 full, 2 mixed}  (kb = kv block, qb = q block)
      mtile_idx[kb][qb] -> index into mtiles for non-full blocks
      mtiles: [U, 128, 128] bf16 ADDITIVE tiles (-30000 where masked),
              already transposed to [kv_local, q_local]
    """
    kind = [[0] * NB for _ in range(NB)]
    idx = [[-1] * NB for _ in range(NB)]
    uniq = {}
    tiles = []

    def intern(blk):
        tT = np.ascontiguousarray(
            (MASK_NEG * (1.0 - blk.T.astype(np.float32))).astype(BF16))
        key = tT.tobytes()
        if key not in uniq:
            uniq[key] = len(tiles)
            tiles.append(tT)
        return uniq[key]

    for kb in range(NB):
        for qb in range(NB):
            blk = m2[qb * 128:(qb + 1) * 128, kb * 128:(kb + 1) * 128]
            s = int(blk.sum())
            if s == 0:
                kind[kb][qb] = 0
            elif s == 128 * 128:
                kind[kb][qb] = 1
            else:
                kind[kb][qb] = 2
                idx[kb][qb] = intern(blk)
    if len(tiles) > 30:
        raise ValueError(f"mask has {len(tiles)} unique non-full 128x128 "
                         "blocks; kernel supports <= 30")
    return kind, idx, tiles, uniq


# --------------------------------------------------------------------------
# bass kernel builder
# --------------------------------------------------------------------------

def _build_nc(kind, mtile_idx, n_mtiles, zero_u):
    import concourse.bass as bass
    import concourse.mybir as mybir
    import concourse.tile as tile
    from concourse.tile_rust import add_dep_helper

    f32 = mybir.dt.float32
    bf16 = mybir.dt.bfloat16
    f8e4 = mybir.dt.float8e4
    u8 = mybir.dt.uint8
    AF = mybir.ActivationFunctionType
    DR = mybir.MatmulPerfMode.DoubleRow
    MULT = mybir.AluOpType.mult

    nc = bass.Bass()
    # host-transposed inputs, already in SBUF layout:
    # xq8T/xk8T [p, c, j, s] = x[s, 256c + 2p + j] as fp8 bytes
    # xv8T [p, d, s] = xv[s, 128d + p] fp8 bytes
    xq8T = nc.declare_dram_parameter("xq8T", [128, QC, DC2, 2, 512], u8,
                                     isOutput=False)
    xk8T = nc.declare_dram_parameter("xk8T", [128, QC, DC2, 2, 512], u8,
                                     isOutput=False)
    xvT = nc.declare_dram_parameter("xvT", [128, QC, DCH, 512], bf16,
                                    isOutput=False)
    # fp8 DoubleRow weights, og-major: [p, og, c, j, m] = W[og*128 + m,
    # 256c + 2p + j] -- split by head-group so the first chain's weights
    # are only 128KB of critical DMA
    w8q = nc.declare_dram_parameter("w8q", [128, 2, DC2, 2, 128], u8,
                                    isOutput=False)
    w8k = nc.declare_dram_parameter("w8k", [128, 2, DC2, 2, 128], u8,
                                    isOutput=False)
    wvp = nc.declare_dram_parameter("wvp", [128, DCH, OC], bf16,
                                    isOutput=False)
    bq2 = nc.declare_dram_parameter("bq2", [128, 2], f32, isOutput=False)
    bk2 = nc.declare_dram_parameter("bk2", [128, 2], f32, isOutput=False)
    bvr = nc.declare_dram_parameter("bvr", [128, OC], f32, isOutput=False)
    mtd = nc.declare_dram_parameter("mtiles", [128, n_mtiles, 128], bf16,
                                    isOutput=False)
    eyed = nc.declare_dram_parameter("eye", [128, 128], bf16, isOutput=False)
    out = nc.declare_dram_parameter("out", [S, OC], bf16, isOutput=True)

    # last unmasked kv block per q block (for matmul stop flags)
    last_kb = [max((kb for kb in range(NB) if kind[kb][qb]), default=-1)
               for qb in range(NB)]
    # kv blocks needed per q chunk
    kbs_for_qc = [
        [kb for kb in range(NB)
         if any(kind[kb][4 * qc + j] for j in range(4))]
        for qc in range(QC)
    ]

    def first_j(qc, kb):
        for j in range(4):
            if kind[kb][4 * qc + j]:
                return j
        return 4

    with tile.TileContext(nc) as tc:
        with (
            tc.tile_pool(name="consts", bufs=1) as cp,
            tc.tile_pool(name="weights", bufs=1) as wp,
            tc.tile_pool(name="persist", bufs=1) as pp,
            tc.tile_pool(name="xt", bufs=1) as xtp,
            tc.tile_pool(name="ptile", bufs=3) as ptp,
            tc.tile_pool(name="stage", bufs=3) as sgp,
            tc.tile_pool(name="small", bufs=8) as stp,
        ):
            # ---- table-load hider: tiny exp early while DMAs stream ----
            dum = cp.tile([128, 8], f32, tag="dum")
            nc.vector.memset(dum, 0.0)
            dume = cp.tile([128, 8], bf16, tag="dume")
            nc.scalar.activation(out=dume, in_=dum, func=AF.Exp, scale=1.0)

            # ---- constants / weights.  sync + scalar are HW DMA queues
            # (~150 GB/s each); gpsimd is SW-DGE (~35 GB/s) and only gets
            # latency-tolerant loads.  Queue heads carry exactly what the
            # first projection chains need.
            b_sb = {}
            for name, dram in (("q", bq2), ("k", bk2)):
                t = cp.tile([128, 2], f32, tag=f"b{name}")
                nc.scalar.dma_start(out=t, in_=dram[:])
                b_sb[name] = t
            mt_sb = cp.tile([128, n_mtiles, 128], bf16, tag="mt")
            nc.scalar.dma_start(out=mt_sb, in_=mtd[:])
            eye_sb = cp.tile([128, 128], bf16, tag="eye")
            nc.scalar.dma_start(out=eye_sb, in_=eyed[:])
            w8_sb = {}
            w8_q = {"k": nc.sync, "q": nc.scalar}
            w8_dram = {"k": w8k, "q": w8q}
            for name in ("k", "q"):
                t = wp.tile([128, 2, DC2, 2, 128], u8, tag=f"w8{name}")
                w8_q[name].dma_start(out=t[:, 0], in_=w8_dram[name][:, 0])
                w8_sb[name] = t.bitcast(f8e4)
            bv_sb = cp.tile([128, OC], f32, tag="bv")
            wv_sb = wp.tile([128, DCH, OC], bf16, tag="wv")
            nc.gpsimd.dma_start(out=wv_sb, in_=wvp[:])

            # persistent projected tensors
            qT_sb = pp.tile([128, 2, S], bf16, tag="qT")   # [o_local, og, s]
            kT_sb = pp.tile([128, 2, S], bf16, tag="kT")
            v_sb = pp.tile([128, NB * NH, HD + 1], bf16, tag="v")
            nc.vector.memset(v_sb[:, :, HD:HD + 1], 1.0)
            # warm-up operand: zeros, no input dependency
            wu_sb = cp.tile([128, 512], bf16, tag="wu")
            nc.vector.memset(wu_sb, 0.0)

            # ---- input loads ----
            xt8 = {}
            tiles8 = {}
            for name in ("k", "q"):
                t = xtp.tile([128, QC, DC2, 2, 512], u8, tag=f"xt8{name}",
                             name=f"xt8_{name}")
                tiles8[name] = t
                xt8[name] = t.bitcast(f8e4)
            xtv = xtp.tile([128, QC, DCH, 512], bf16, tag="xtv", name="xt_v")
            # sc0 k/q split by contraction-chunk pairs so the first
            # projection chains start on partial data
            for half in range(2):
                cs = slice(2 * half, 2 * half + 2)
                nc.sync.dma_start(out=tiles8["k"][:, 0, cs],
                                  in_=xk8T[:, 0, cs])
                nc.scalar.dma_start(out=tiles8["q"][:, 0, cs],
                                    in_=xq8T[:, 0, cs])
            # og1 weight halves + latency-tolerant consts after the heads
            for name in ("k", "q"):
                w8_q[name].dma_start(out=w8_sb[name].bitcast(u8)[:, 1],
                                     in_=w8_dram[name][:, 1])
            nc.scalar.dma_start(out=bv_sb, in_=bvr[:])
            nc.sync.dma_start(out=xtv[:, 0], in_=xvT[:, 0])
            for sc in range(1, QC):
                nc.sync.dma_start(out=tiles8["k"][:, sc], in_=xk8T[:, sc])
                nc.scalar.dma_start(out=tiles8["q"][:, sc], in_=xq8T[:, sc])
                if sc < 3:
                    nc.sync.dma_start(out=xtv[:, sc], in_=xvT[:, sc])
            nc.gpsimd.dma_start(out=xtv[:, 3], in_=xvT[:, 3])

            # ---- projection building blocks (shared psum pool) ----
            with tc.tile_pool(name="dfp", bufs=2, space="PSUM") as dfp:
                def qk_chain(name, og, sc):
                    ps = dfp.tile([128, 512], f32, tag="dps",
                                  name=f"ps_{name}{og}{sc}")
                    for c in range(DC2):
                        nc.tensor.matmul(
                            ps,
                            w8_sb[name][:, og, c],
                            xt8[name][:, sc, c],
                            start=(c == 0), stop=(c == DC2 - 1),
                            perf_mode=DR)
                    dst = kT_sb if name == "k" else qT_sb
                    nc.vector.tensor_scalar_add(
                        dst[:, og, sc * 512:(sc + 1) * 512], ps,
                        b_sb[name][:, og:og + 1])

                def v_group(g):
                    # V-proj for row-block pair (2g, 2g+1): d-inner so the
                    # psum tile completes immediately (1 live bank)
                    vps = dfp.tile([128, 2, OC], f32, tag="dps",
                                   name=f"vps{g}")
                    first = [None]
                    for d in range(DCH):
                        for i, sb in enumerate((2 * g, 2 * g + 1)):
                            mm = nc.tensor.matmul(
                                vps[:, i, :],
                                xtv[:, sb // 4, d,
                                    (sb % 4) * 128:(sb % 4 + 1) * 128],
                                wv_sb[:, d, :],
                                start=(d == 0 and i == 0),
                                stop=(d == DCH - 1 and i == 1),
                                skip_group_check=True)
                            if d == 0 and i == 0:
                                first[0] = mm
                            elif d == 0:
                                add_dep_helper(
                                    mm.ins, first[0].ins, sync=False,
                                    reason="psum bank clear before packed "
                                           "write")
                    # bv folded into v rows: (vproj + bv) -> bf16
                    for i, sb in enumerate((2 * g, 2 * g + 1)):
                        nc.vector.tensor_add(
                            v_sb[:, sb * NH:(sb + 1) * NH, 0:HD],
                            vps[:, i, :].rearrange("p (h d) -> p h d", h=NH),
                            bv_sb.rearrange("p (h d) -> p h d", h=NH))

                # PE warm-up: dummy matmuls with no input deps so HAM is at
                # full clock by the time the first projection chain runs
                with tc.tile_pool(name="wups", bufs=1, space="PSUM") as wup:
                    wps = wup.tile([128, 512], f32, tag="wups")
                    for i in range(24):
                        nc.tensor.matmul(wps, wu_sb[:, 0:128], wu_sb,
                                         start=True, stop=True,
                                         skip_group_check=True)

                    # upfront projections: only what (qc0, og0) needs
                    qk_chain("k", 0, 0)
                    qk_chain("q", 0, 0)

                # deferred units dripped through attention, keyed by global
                # kb-iteration index (popped after each iteration's AV;
                # deadline: ready before first use)
                deferred = {}
                sched = [
                    (0, "k", 1, 0), (1, "q", 1, 0), (2, "v", 0, 0),
                    (3, "v", 1, 0), (5, "q", 0, 1), (6, "k", 0, 1),
                    (9, "v", 2, 0), (10, "q", 1, 1), (11, "v", 3, 0),
                    (13, "k", 1, 1), (20, "q", 0, 2), (22, "k", 0, 2),
                    (26, "v", 4, 0), (28, "v", 5, 0), (31, "q", 1, 2),
                    (34, "k", 1, 2), (40, "q", 0, 3), (44, "k", 0, 3),
                    (50, "v", 6, 0), (54, "v", 7, 0), (58, "q", 1, 3),
                    (62, "k", 1, 3),
                ]
                for it, kindu, a, b in sched:
                    deferred.setdefault(it, []).append((kindu, a, b))

                # ---- attention ----
                with (
                    tc.tile_pool(name="stps", bufs=2, space="PSUM") as sp,
                    tc.tile_pool(name="ops", bufs=2, space="PSUM") as op,
                ):
                    git = 0  # global kb-iteration counter
                    for qc in range(QC):
                        # interleave full and mixed kv blocks: a mixed block's
                        # longer score+mask chain latency hides under the
                        # preceding full block's wide exp activation
                        kraw = kbs_for_qc[qc]
                        fulls = [kb for kb in kraw
                                 if all(kind[kb][4 * qc + j] == 1
                                        for j in range(4))]
                        mixeds = [kb for kb in kraw if kb not in fulls]
                        kbs = []
                        for i in range(max(len(fulls), len(mixeds))):
                            if i < len(fulls):
                                kbs.append(fulls[i])
                            if i < len(mixeds):
                                kbs.append(mixeds[i])
                        stage = sgp.tile([128, 4, OC], bf16, tag="stage")
                        for og in range(2):
                            o_blk = [
                                op.tile([128, 4, HD + 1], f32, tag="ops",
                                        name=f"o{qc}{og}{ph}")
                                for ph in range(2)
                            ]
                            started = [[False] * 4, [False] * 4]
                            clear_mm = [None, None]
                            for kb in kbs:
                                fj = first_j(qc, kb)
                                off = 128 * fj
                                st2 = sp.tile([128, 2, 512], f32, tag="stps")
                                mixed = any(
                                    kind[kb][4 * qc + j] != 1
                                    for j in range(fj, 4))
                                for ph in range(2):
                                    nc.tensor.matmul(
                                        st2[:, ph, off:512],
                                        kT_sb[ph * 64:(ph + 1) * 64, og,
                                              kb * 128:(kb + 1) * 128],
                                        qT_sb[ph * 64:(ph + 1) * 64, og,
                                              qc * 512 + off:(qc + 1) * 512],
                                        start=True, stop=(not mixed),
                                        skip_group_check=True)
                                if mixed:
                                    for j in range(fj, 4):
                                        bk = kind[kb][4 * qc + j]
                                        if bk == 1:
                                            continue
                                        u = (mtile_idx[kb][4 * qc + j]
                                             if bk == 2 else zero_u)
                                        lastm = (j == max(
                                            jj for jj in range(fj, 4)
                                            if kind[kb][4 * qc + jj] != 1))
                                        for ph in range(2):
                                            nc.tensor.matmul(
                                                st2[:, ph,
                                                    j * 128:(j + 1) * 128],
                                                eye_sb,
                                                mt_sb[:, u, :],
                                                start=False,
                                                stop=lastm,
                                                skip_group_check=True)
                                pt = ptp.tile([128, 2, 512], bf16, tag="pt")
                                nc.scalar.activation(
                                    out=pt[:, :, off:512],
                                    in_=st2[:, :, off:512],
                                    func=AF.Exp, scale=SCALE)
                                for ph in range(2):
                                    h = 2 * og + ph
                                    for j in range(fj, 4):
                                        qb = 4 * qc + j
                                        if kind[kb][qb] == 0:
                                            continue
                                        mm = nc.tensor.matmul(
                                            o_blk[ph][:, j, :],
                                            pt[:, ph, j * 128:(j + 1) * 128],
                                            v_sb[:, kb * NH + h, :],
                                            start=(clear_mm[ph] is None),
                                            stop=(kb == last_kb[qb]),
                                            skip_group_check=True)
                                        if clear_mm[ph] is None:
                                            clear_mm[ph] = mm
                                        elif not started[ph][j]:
                                            add_dep_helper(
                                                mm.ins, clear_mm[ph].ins,
                                                sync=False,
                                                reason="psum bank clear "
                                                       "before packed write")
                                        started[ph][j] = True
                                for unit in deferred.pop(git, ()):
                                    ku, a, b = unit
                                    if ku == "v":
                                        v_group(a)
                                    else:
                                        qk_chain(ku, a, b)
                                git += 1
                            # normalize both heads of the pair
                            for ph in range(2):
                                h = 2 * og + ph
                                rec = stp.tile([128, 4, 1], f32, tag="rec")
                                nc.vector.reciprocal(
                                    rec, o_blk[ph][:, :, HD:HD + 1])
                                nc.vector.tensor_tensor(
                                    stage[:, :, h * HD:(h + 1) * HD],
                                    o_blk[ph][:, :, 0:HD],
                                    rec.broadcast_to([128, 4, HD]),
                                    MULT)
                            # store this head-pair's half as soon as ready
                            nc.sync.dma_start(
                                out=out[qc * 512:(qc + 1) * 512,
                                        og * 128:(og + 1) * 128]
                                .rearrange("(j p) o -> p j o", p=128),
                                in_=stage[:, :, og * 128:(og + 1) * 128])
    return nc


# --------------------------------------------------------------------------
# entry point
# --------------------------------------------------------------------------

def kernel(qx, kx, vx, mask, Wq, bq, Wk, bk, Wv, bv):
    _install_shims()
    from concourse.bass_utils import run_bass_kernel_spmd

    qx = np.asarray(qx)
    kx = np.asarray(kx)
    vx = np.asarray(vx)
    mask = np.asarray(mask)
    Wq = np.asarray(Wq, np.float32)
    bq = np.asarray(bq, np.float32)
    Wk = np.asarray(Wk, np.float32)
    bk = np.asarray(bk, np.float32)
    Wv = np.asarray(Wv, np.float32)
    bv = np.asarray(bv, np.float32)

    m2 = mask.reshape(S, S)
    kind, mtile_idx, tiles, uniq = _classify_mask(m2)

    # ensure an all-masked additive tile exists for kind-0 blocks inside a
    # trimmed diagonal span
    def first_j(qc, kb):
        for j in range(4):
            if kind[kb][4 * qc + j]:
                return j
        return 4

    zero_u = -1
    for qc in range(QC):
        for kb in range(NB):
            fj = first_j(qc, kb)
            if fj >= 4:
                continue
            for j in range(fj, 4):
                if kind[kb][4 * qc + j] == 0:
                    allm = np.full((128, 128), np.float32(MASK_NEG))
                    tT = np.ascontiguousarray(allm.astype(BF16))
                    key = tT.tobytes()
                    if key not in uniq:
                        uniq[key] = len(tiles)
                        tiles.append(tT)
                    zero_u = uniq[key]
    if not tiles:
        tiles.append(np.zeros((128, 128), BF16))
    mtiles = np.stack(tiles)

    key = (tuple(tuple(r) for r in kind),
           tuple(tuple(r) for r in mtile_idx), mtiles.shape[0], zero_u)
    if key not in _cache:
        _cache[key] = _build_nc(kind, mtile_idx, mtiles.shape[0], zero_u)
    nc = _cache[key]

    def w8_pack(W, sl):
        # [p, og, c, j, m] = W[sl][128*og + m, 256c + 2p + j] as fp8 bytes
        w = W[sl].astype(F8)                       # [256, 1024]
        w = w.T.reshape(DC2, 128, 2, 2, 128)       # [c, p, j, og, m]
        return np.ascontiguousarray(
            w.transpose(1, 3, 0, 2, 4)).view(np.uint8)

    def x8T(x):
        # [p, sc, c, j, s] = x[512*sc + s, 256c + 2p + j]
        t = x.astype(F8).reshape(QC, 512, DC2, 128, 2)
        return np.ascontiguousarray(t.transpose(3, 0, 2, 4, 1)).view(np.uint8)

    def xvT_pack(x):
        # [p, sc, d, s] = x[512*sc + s, 128d + p]
        t = x.astype(BF16).reshape(QC, 512, DCH, 128)
        return np.ascontiguousarray(t.transpose(3, 0, 2, 1))

    eye = np.ascontiguousarray(np.eye(128, dtype=np.float32).astype(BF16))

    xb = {b: (x8T(qx[b]), x8T(kx[b]), xvT_pack(vx[b])) for b in range(B)}
    in_maps = []
    for c in range(8):
        b, hg = divmod(c, 4)
        sl = slice(hg * OC, (hg + 1) * OC)
        in_maps.append({
            "xq8T": xb[b][0],
            "xk8T": xb[b][1],
            "xvT": xb[b][2],
            "w8q": w8_pack(Wq, sl),
            "w8k": w8_pack(Wk, sl),
            "wvp": np.ascontiguousarray(
                Wv[sl].T.astype(BF16).reshape(DCH, 128, OC)
                .transpose(1, 0, 2)),
            "bq2": np.ascontiguousarray(bq[sl].reshape(2, 128).T,
                                        dtype=np.float32),
            "bk2": np.ascontiguousarray(bk[sl].reshape(2, 128).T,
                                        dtype=np.float32),
            "bvr": np.ascontiguousarray(
                np.broadcast_to(bv[sl].astype(np.float32), (128, OC))),
            "mtiles": np.ascontiguousarray(mtiles.transpose(1, 0, 2)),
            "eye": eye,
        })

    trace = os.environ.get("BASS_KERNEL_TRACE") == "1"
    if trace:
        # warm run first: profiling start before the first executable load
        # wedges the load under axon
        run_bass_kernel_spmd(nc, in_maps, list(range(8)), trace=False)
    res = run_bass_kernel_spmd(nc, in_maps, list(range(8)), trace=trace)
    if trace:
        print(f"HW exec time: {res.exec_time_ns} ns "
              f"(mean {res.mean_exec_time_ns})")

    outp = np.zeros((B, S, D), np.float32)
    for c in range(8):
        b, hg = divmod(c, 4)
        outp[b, :, hg * OC:(hg + 1) * OC] = np.asarray(
            res.results[c]["out"]).astype(np.float32)
    return outp


# revision 36
# speedup vs baseline: 1.0706x; 1.0706x over previous
"""Trainium2 Bass kernel for nn_Decoder_26585847562664 (v3).

16-head causal attention decoder: B=2, S=2048, D=1024, HD=64.
Sharded over 8 NeuronCores as (batch x head-group): core c handles batch
c//4 and heads [4*(c%4), 4*(c%4)+4) -- Wq/Wk/Wv are split column-wise by
head group on the host, so every core's work is fully local (no
collectives).

v3 changes vs v2:
- Scores for a head PAIR are computed by two row-tiled concurrent K=64
  matmuls (partitions 0-63 / 64-127) into one [128, 2, 512] psum tile;
  ONE exp activation covers both heads (80 activations, fewer+wider).
- Causal mask applied additively PRE-exp by an identity-weight matmul
  adding -30000*(1-mask) into score psum; all DVE mask multiplies gone.
- bv folded into v_sb rows (numerator absorbs Z*bv), so normalization
  needs no bias add; reciprocals batched [128, 4] per head.
- Q/K bias psum->sbuf moves on DVE (tensor_scalar add); the Scalar
  engine runs exp only.
- V activations arrive in fp8 (halves the largest input DMA); V
  projection is fp8 stationary x bf16 moving.
- Projection chains drip through attention on a deadline schedule to
  keep the PE HAM-warm.
"""

import json
import os
import sys
import types

import numpy as np
import ml_dtypes

B, S, D, H = 2, 2048, 1024, 16
HD = 64
NH = 4            # heads per core
OC = NH * HD      # 256 projection columns per core
NB = S // 128     # 16 row blocks
QC = S // 512     # 4 q-chunks of 512
DCH = D // 128    # 8 contraction chunks (bf16 path)
DC2 = D // 256    # 4 fused contraction chunks (fp8 DoubleRow path)
SCALE = 1.0 / 32.0  # 1/sqrt(D)
MASK_NEG = -30000.0

BF16 = ml_dtypes.bfloat16
F8 = ml_dtypes.float8_e4m3

_cache = {}


# --------------------------------------------------------------------------
# environment shims (walrus single-wait limit, missing NTFF hook, no egress)
# --------------------------------------------------------------------------

def _install_shims():
    import concourse.bass as bass

    if not getattr(bass.Bass.to_json_bytes, "_wait_split", False):
        orig = bass.Bass.to_json_bytes

        def to_json_bytes(self):
            m = json.loads(orig(self))
            for fn in m.get("functions", []):
                for bb in fn.get("blocks", []):
                    out = []
                    for inst in bb.get("instructions", []):
                        si = inst.get("sync_info")
                        waits = (si or {}).get("on_wait") or []
                        if len(waits) > 1:
                            for j, w in enumerate(waits[:-1]):
                                out.append({
                                    "debug": inst.get("debug", 0),
                                    "engine": inst["engine"],
                                    "ins": [],
                                    "name": f"{inst['name']}ws{j}",
                                    "opcode": "NoOp",
                                    "outs": [],
                                    "sync_info": {"on_update": [], "on_wait": [w]},
                                })
                            si["on_wait"] = [waits[-1]]
                        out.append(inst)
                    bb["instructions"] = out
            return json.dumps(m).encode()

        to_json_bytes._wait_split = True
        bass.Bass.to_json_bytes = to_json_bytes

    try:
        import antenv
        try:
            from antenv import axon_hooks  # noqa: F401
        except ImportError:
            from trn_agent_boot.trn_boot import _ntff_profile_via_ctypes

            mod = types.ModuleType("antenv.axon_hooks")
            hook = [_ntff_profile_via_ctypes("/opt/axon/libaxon_pjrt.so")]
            mod.get_axon_ntff_profile_hook = lambda: hook[0]
            mod.set_axon_ntff_profile_hook = lambda h: hook.__setitem__(0, h)
            sys.modules["antenv.axon_hooks"] = mod
            antenv.axon_hooks = mod
    except Exception:
        pass

    try:
        from concourse import bass_utils
        bass_utils.upload_artifacts = lambda tmpdir: "local://skipped"
    except Exception:
        pass


# --------------------------------------------------------------------------
# mask block classification (host side)
# --------------------------------------------------------------------------

def _classify_mask(m2):
    """m2: [S, S] int array, m2[q, k] == 1 -> position attended.

    Returns (kind, mtile_idx, mtiles):
      kind[kb][qb]  in {0 zero, 1 full, 2 mixed}  (kb = kv block, qb = q block)
      mtile_idx[kb][qb] -> index into mtiles for non-full blocks
      mtiles: [U, 128, 128] bf16 ADDITIVE tiles (-30000 where masked),
              already transposed to [kv_local, q_local]
    """
    kind = [[0] * NB for _ in range(NB)]
    idx = [[-1] * NB for _ in range(NB)]
    uniq = {}
    tiles = []

    def intern(blk):
        tT = np.ascontiguousarray(
            (MASK_NEG * (1.0 - blk.T.astype(np.float32))).astype(BF16))
        key = tT.tobytes()
        if key not in uniq:
            uniq[key] = len(tiles)
            tiles.append(tT)
        return uniq[key]

    for kb in range(NB):
        for qb in range(NB):
            blk = m2[qb * 128:(qb + 1) * 128, kb * 128:(kb + 1) * 128]
            s = int(blk.sum())
            if s == 0:
                kind[kb][qb] = 0
            elif s == 128 * 128:
                kind[kb][qb] = 1
            else:
                kind[kb][qb] = 2
                idx[kb][qb] = intern(blk)
    if len(tiles) > 30:
        raise ValueError(f"mask has {len(tiles)} unique non-full 128x128 "
                         "blocks; kernel supports <= 30")
    return kind, idx, tiles, uniq


# --------------------------------------------------------------------------
# bass kernel builder
# --------------------------------------------------------------------------

def _build_nc(kind, mtile_idx, n_mtiles, zero_u):
    import concourse.bass as bass
    import concourse.mybir as mybir
    import concourse.tile as tile
    from concourse.tile_rust import add_dep_helper

    f32 = mybir.dt.float32
    bf16 = mybir.dt.bfloat16
    f8e4 = mybir.dt.float8e4
    u8 = mybir.dt.uint8
    AF = mybir.ActivationFunctionType
    DR = mybir.MatmulPerfMode.DoubleRow
    MULT = mybir.AluOpType.mult

    nc = bass.Bass()
    # host-transposed inputs, already in SBUF layout:
    # xq8T/xk8T [p, c, j, s] = x[s, 256c + 2p + j] as fp8 bytes
    # xv8T [p, d, s] = xv[s, 128d + p] fp8 bytes
    xq8T = nc.declare_dram_parameter("xq8T", [128, QC, DC2, 2, 512], u8,
                                     isOutput=False)
    xk8T = nc.declare_dram_parameter("xk8T", [128, QC, DC2, 2, 512], u8,
                                     isOutput=False)
    xvT = nc.declare_dram_parameter("xvT", [128, QC, DCH, 512], bf16,
                                    isOutput=False)
    # fp8 DoubleRow weights, og-major: [p, og, c, j, m] = W[og*128 + m,
    # 256c + 2p + j] -- split by head-group so the first chain's weights
    # are only 128KB of critical DMA
    w8q = nc.declare_dram_parameter("w8q", [128, 2, DC2, 2, 128], u8,
                                    isOutput=False)
    w8k = nc.declare_dram_parameter("w8k", [128, 2, DC2, 2, 128], u8,
                                    isOutput=False)
    wvp = nc.declare_dram_parameter("wvp", [128, DCH, OC], bf16,
                                    isOutput=False)
    bq2 = nc.declare_dram_parameter("bq2", [128, 2], f32, isOutput=False)
    bk2 = nc.declare_dram_parameter("bk2", [128, 2], f32, isOutput=False)
    bvr = nc.declare_dram_parameter("bvr", [128, OC], f32, isOutput=False)
    mtd = nc.declare_dram_parameter("mtiles", [128, n_mtiles, 128], bf16,
                                    isOutput=False)
    eyed = nc.declare_dram_parameter("eye", [128, 128], bf16, isOutput=False)
    # DMA-friendly store layout: [qc, og, p, j, oc_half]; per partition each
    # store writes one contiguous 1KB run (the host undoes the permutation)
    out = nc.declare_dram_parameter("out", [QC, 2, 128, 4, 128], bf16,
                                    isOutput=True)

    # last unmasked kv block per q block (for matmul stop flags)
    last_kb = [max((kb for kb in range(NB) if kind[kb][qb]), default=-1)
               for qb in range(NB)]
    # kv blocks needed per q chunk
    kbs_for_qc = [
        [kb for kb in range(NB)
         if any(kind[kb][4 * qc + j] for j in range(4))]
        for qc in range(QC)
    ]

    def first_j(qc, kb):
        for j in range(4):
            if kind[kb][4 * qc + j]:
                return j
        return 4

    with tile.TileContext(nc) as tc:
        with (
            tc.tile_pool(name="consts", bufs=1) as cp,
            tc.tile_pool(name="weights", bufs=1) as wp,
            tc.tile_pool(name="persist", bufs=1) as pp,
            tc.tile_pool(name="xt", bufs=1) as xtp,
            tc.tile_pool(name="ptile", bufs=3) as ptp,
            tc.tile_pool(name="stage", bufs=3) as sgp,
            tc.tile_pool(name="small", bufs=8) as stp,
        ):
            # ---- table-load hider: tiny exp early while DMAs stream ----
            dum = cp.tile([128, 8], f32, tag="dum")
            nc.vector.memset(dum, 0.0)
            dume = cp.tile([128, 8], bf16, tag="dume")
            nc.scalar.activation(out=dume, in_=dum, func=AF.Exp, scale=1.0)

            # ---- constants / weights.  sync + scalar are HW DMA queues
            # (~150 GB/s each); gpsimd is SW-DGE (~35 GB/s) and only gets
            # latency-tolerant loads.  Queue heads carry exactly what the
            # first projection chains need.
            b_sb = {}
            for name, dram in (("q", bq2), ("k", bk2)):
                t = cp.tile([128, 2], f32, tag=f"b{name}")
                nc.scalar.dma_start(out=t, in_=dram[:])
                b_sb[name] = t
            mt_sb = cp.tile([128, n_mtiles, 128], bf16, tag="mt")
            nc.scalar.dma_start(out=mt_sb, in_=mtd[:])
            eye_sb = cp.tile([128, 128], bf16, tag="eye")
            nc.scalar.dma_start(out=eye_sb, in_=eyed[:])
            w8_sb = {}
            w8_q = {"k": nc.sync, "q": nc.scalar}
            w8_dram = {"k": w8k, "q": w8q}
            for name in ("k", "q"):
                t = wp.tile([128, 2, DC2, 2, 128], u8, tag=f"w8{name}")
                w8_q[name].dma_start(out=t[:, 0], in_=w8_dram[name][:, 0])
                w8_sb[name] = t.bitcast(f8e4)
            bv_sb = cp.tile([128, OC], f32, tag="bv")
            wv_sb = wp.tile([128, DCH, OC], bf16, tag="wv")
            nc.gpsimd.dma_start(out=wv_sb, in_=wvp[:])

            # persistent projected tensors
            qT_sb = pp.tile([128, 2, S], bf16, tag="qT")   # [o_local, og, s]
            kT_sb = pp.tile([128, 2, S], bf16, tag="kT")
            v_sb = pp.tile([128, NB * NH, HD + 1], bf16, tag="v")
            nc.vector.memset(v_sb[:, :, HD:HD + 1], 1.0)
            # warm-up operand: zeros, no input dependency
            wu_sb = cp.tile([128, 512], bf16, tag="wu")
            nc.vector.memset(wu_sb, 0.0)

            # ---- input loads ----
            xt8 = {}
            tiles8 = {}
            for name in ("k", "q"):
                t = xtp.tile([128, QC, DC2, 2, 512], u8, tag=f"xt8{name}",
                             name=f"xt8_{name}")
                tiles8[name] = t
                xt8[name] = t.bitcast(f8e4)
            xtv = xtp.tile([128, QC, DCH, 512], bf16, tag="xtv", name="xt_v")
            # sc0 k/q split by contraction-chunk pairs so the first
            # projection chains start on partial data
            for half in range(2):
                cs = slice(2 * half, 2 * half + 2)
                nc.sync.dma_start(out=tiles8["k"][:, 0, cs],
                                  in_=xk8T[:, 0, cs])
                nc.scalar.dma_start(out=tiles8["q"][:, 0, cs],
                                    in_=xq8T[:, 0, cs])
            # og1 weight halves + latency-tolerant consts after the heads
            for name in ("k", "q"):
                w8_q[name].dma_start(out=w8_sb[name].bitcast(u8)[:, 1],
                                     in_=w8_dram[name][:, 1])
            nc.scalar.dma_start(out=bv_sb, in_=bvr[:])
            nc.sync.dma_start(out=xtv[:, 0], in_=xvT[:, 0])
            for sc in range(1, QC):
                nc.sync.dma_start(out=tiles8["k"][:, sc], in_=xk8T[:, sc])
                nc.scalar.dma_start(out=tiles8["q"][:, sc], in_=xq8T[:, sc])
                if sc < 3:
                    nc.sync.dma_start(out=xtv[:, sc], in_=xvT[:, sc])
            nc.gpsimd.dma_start(out=xtv[:, 3], in_=xvT[:, 3])

            # ---- projection building blocks (shared psum pool) ----
            with tc.tile_pool(name="dfp", bufs=2, space="PSUM") as dfp:
                def qk_chain(name, og, sc):
                    ps = dfp.tile([128, 512], f32, tag="dps",
                                  name=f"ps_{name}{og}{sc}")
                    for c in range(DC2):
                        nc.tensor.matmul(
                            ps,
                            w8_sb[name][:, og, c],
                            xt8[name][:, sc, c],
                            start=(c == 0), stop=(c == DC2 - 1),
                            perf_mode=DR)
                    dst = kT_sb if name == "k" else qT_sb
                    nc.vector.tensor_scalar_add(
                        dst[:, og, sc * 512:(sc + 1) * 512], ps,
                        b_sb[name][:, og:og + 1])

                def v_group(g):
                    # V-proj for row-block pair (2g, 2g+1): d-inner so the
                    # psum tile completes immediately (1 live bank)
                    vps = dfp.tile([128, 2, OC], f32, tag="dps",
                                   name=f"vps{g}")
                    first = [None]
                    for d in range(DCH):
                        for i, sb in enumerate((2 * g, 2 * g + 1)):
                            mm = nc.tensor.matmul(
                                vps[:, i, :],
                                xtv[:, sb // 4, d,
                                    (sb % 4) * 128:(sb % 4 + 1) * 128],
                                wv_sb[:, d, :],
                                start=(d == 0 and i == 0),
                                stop=(d == DCH - 1 and i == 1),
                                skip_group_check=True)
                            if d == 0 and i == 0:
                                first[0] = mm
                            elif d == 0:
                                add_dep_helper(
                                    mm.ins, first[0].ins, sync=False,
                                    reason="psum bank clear before packed "
                                           "write")
                    # bv folded into v rows: (vproj + bv) -> bf16
                    for i, sb in enumerate((2 * g, 2 * g + 1)):
                        nc.vector.tensor_add(
                            v_sb[:, sb * NH:(sb + 1) * NH, 0:HD],
                            vps[:, i, :].rearrange("p (h d) -> p h d", h=NH),
                            bv_sb.rearrange("p (h d) -> p h d", h=NH))

                # PE warm-up: dummy matmuls with no input deps so HAM is at
                # full clock by the time the first projection chain runs
                with tc.tile_pool(name="wups", bufs=1, space="PSUM") as wup:
                    wps = wup.tile([128, 512], f32, tag="wups")
                    for i in range(24):
                        nc.tensor.matmul(wps, wu_sb[:, 0:128], wu_sb,
                                         start=True, stop=True,
                                         skip_group_check=True)

                    # upfront projections: both head-pairs' sc0 chains (the
                    # xq0 DMA gates attention start either way; og1 chains
                    # ride along warm behind og0's)
                    qk_chain("k", 0, 0)
                    qk_chain("k", 1, 0)
                    qk_chain("q", 0, 0)
                    qk_chain("q", 1, 0)

                # deferred units dripped through attention, keyed by global
                # kb-iteration index (popped after each iteration's AV;
                # deadline: ready before first use)
                deferred = {}
                sched = [
                    (0, "v", 0, 0), (1, "v", 1, 0), (3, "q", 0, 1),
                    (5, "k", 0, 1), (7, "v", 2, 0), (9, "v", 3, 0),
                    (10, "q", 1, 1), (13, "k", 1, 1), (18, "q", 0, 2),
                    (22, "k", 0, 2), (26, "v", 4, 0), (28, "v", 5, 0),
                    (31, "q", 1, 2), (34, "k", 1, 2), (40, "q", 0, 3),
                    (44, "k", 0, 3), (50, "v", 6, 0), (52, "v", 7, 0),
                    (56, "q", 1, 3), (60, "k", 1, 3),
                ]
                for it, kindu, a, b in sched:
                    deferred.setdefault(it, []).append((kindu, a, b))

                # ---- attention ----
                with (
                    tc.tile_pool(name="stps", bufs=2, space="PSUM") as sp,
                    tc.tile_pool(name="ops", bufs=2, space="PSUM") as op,
                ):
                    git = 0  # global kb-iteration counter
                    for qc in range(QC):
                        kbs = kbs_for_qc[qc]
                        stage = sgp.tile([128, 4, OC], bf16, tag="stage")
                        for og in range(2):
                            o_blk = [
                                op.tile([128, 4, HD + 1], f32, tag="ops",
                                        name=f"o{qc}{og}{ph}")
                                for ph in range(2)
                            ]
                            started = [[False] * 4, [False] * 4]
                            clear_mm = [None, None]
                            for kb in kbs:
                                fj = first_j(qc, kb)
                                off = 128 * fj
                                st2 = sp.tile([128, 2, 512], f32, tag="stps")
                                mixed = any(
                                    kind[kb][4 * qc + j] != 1
                                    for j in range(fj, 4))
                                for ph in range(2):
                                    nc.tensor.matmul(
                                        st2[:, ph, off:512],
                                        kT_sb[ph * 64:(ph + 1) * 64, og,
                                              kb * 128:(kb + 1) * 128],
                                        qT_sb[ph * 64:(ph + 1) * 64, og,
                                              qc * 512 + off:(qc + 1) * 512],
                                        start=True, stop=(not mixed),
                                        skip_group_check=True)
                                if mixed:
                                    for j in range(fj, 4):
                                        bk = kind[kb][4 * qc + j]
                                        if bk == 1:
                                            continue
                                        u = (mtile_idx[kb][4 * qc + j]
                                             if bk == 2 else zero_u)
                                        lastm = (j == max(
                                            jj for jj in range(fj, 4)
                                            if kind[kb][4 * qc + jj] != 1))
                                        for ph in range(2):
                                            nc.tensor.matmul(
                                                st2[:, ph,
                                                    j * 128:(j + 1) * 128],
                                                eye_sb,
                                                mt_sb[:, u, :],
                                                start=False,
                                                stop=lastm,
                                                skip_group_check=True)
                                pt = ptp.tile([128, 2, 512], bf16, tag="pt")
                                nc.scalar.activation(
                                    out=pt[:, :, off:512],
                                    in_=st2[:, :, off:512],
                                    func=AF.Exp, scale=SCALE)
                                for ph in range(2):
                                    h = 2 * og + ph
                                    for j in range(fj, 4):
                                        qb = 4 * qc + j
                                        if kind[kb][qb] == 0:
                                            continue
                                        mm = nc.tensor.matmul(
                                            o_blk[ph][:, j, :],
                                            pt[:, ph, j * 128:(j + 1) * 128],
                                            v_sb[:, kb * NH + h, :],
                                            start=(clear_mm[ph] is None),
                                            stop=(kb == last_kb[qb]),
                                            skip_group_check=True)
                                        if clear_mm[ph] is None:
                                            clear_mm[ph] = mm
                                        elif not started[ph][j]:
                                            add_dep_helper(
                                                mm.ins, clear_mm[ph].ins,
                                                sync=False,
                                                reason="psum bank clear "
                                                       "before packed write")
                                        started[ph][j] = True
                                for unit in deferred.pop(git, ()):
                                    ku, a, b = unit
                                    if ku == "v":
                                        v_group(a)
                                    else:
                                        qk_chain(ku, a, b)
                                git += 1
                            # normalize both heads of the pair
                            for ph in range(2):
                                h = 2 * og + ph
                                rec = stp.tile([128, 4, 1], f32, tag="rec")
                                nc.vector.reciprocal(
                                    rec, o_blk[ph][:, :, HD:HD + 1])
                                nc.vector.tensor_tensor(
                                    stage[:, :, h * HD:(h + 1) * HD],
                                    o_blk[ph][:, :, 0:HD],
                                    rec.broadcast_to([128, 4, HD]),
                                    MULT)
                            # store this head-pair's half as soon as ready
                            nc.sync.dma_start(
                                out=out[qc, og],
                                in_=stage[:, :, og * 128:(og + 1) * 128])
    return nc


# --------------------------------------------------------------------------
# entry point
# --------------------------------------------------------------------------

def kernel(qx, kx, vx, mask, Wq, bq, Wk, bk, Wv, bv):
    _install_shims()
    from concourse.bass_utils import run_bass_kernel_spmd

    qx = np.asarray(qx)
    kx = np.asarray(kx)
    vx = np.asarray(vx)
    mask = np.asarray(mask)
    Wq = np.asarray(Wq, np.float32)
    bq = np.asarray(bq, np.float32)
    Wk = np.asarray(Wk, np.float32)
    bk = np.asarray(bk, np.float32)
    Wv = np.asarray(Wv, np.float32)
    bv = np.asarray(bv, np.float32)

    m2 = mask.reshape(S, S)
    kind, mtile_idx, tiles, uniq = _classify_mask(m2)

    # ensure an all-masked additive tile exists for kind-0 blocks inside a
    # trimmed diagonal span
    def first_j(qc, kb):
        for j in range(4):
            if kind[kb][4 * qc + j]:
                return j
        return 4

    zero_u = -1
    for qc in range(QC):
        for kb in range(NB):
            fj = first_j(qc, kb)
            if fj >= 4:
                continue
            for j in range(fj, 4):
                if kind[kb][4 * qc + j] == 0:
                    allm = np.full((128, 128), np.float32(MASK_NEG))
                    tT = np.ascontiguousarray(allm.astype(BF16))
                    key = tT.tobytes()
                    if key not in uniq:
                        uniq[key] = len(tiles)
                        tiles.append(tT)
                    zero_u = uniq[key]
    if not tiles:
        tiles.append(np.zeros((128, 128), BF16))
    mtiles = np.stack(tiles)

    key = (tuple(tuple(r) for r in kind),
           tuple(tuple(r) for r in mtile_idx), mtiles.shape[0], zero_u)
    if key not in _cache:
        _cache[key] = _build_nc(kind, mtile_idx, mtiles.shape[0], zero_u)
    nc = _cache[key]

    def w8_pack(W, sl):
        # [p, og, c, j, m] = W[sl][128*og + m, 256c + 2p + j] as fp8 bytes
        w = W[sl].astype(F8)                       # [256, 1024]
        w = w.T.reshape(DC2, 128, 2, 2, 128)       # [c, p, j, og, m]
        return np.ascontiguousarray(
            w.transpose(1, 3, 0, 2, 4)).view(np.uint8)

    def x8T(x):
        # [p, sc, c, j, s] = x[512*sc + s, 256c + 2p + j]
        t = x.astype(F8).reshape(QC, 512, DC2, 128, 2)
        return np.ascontiguousarray(t.transpose(3, 0, 2, 4, 1)).view(np.uint8)

    def xvT_pack(x):
        # [p, sc, d, s] = x[512*sc + s, 128d + p]
        t = x.astype(BF16).reshape(QC, 512, DCH, 128)
        return np.ascontiguousarray(t.transpose(3, 0, 2, 1))

    eye = np.ascontiguousarray(np.eye(128, dtype=np.float32).astype(BF16))

    xb = {b: (x8T(qx[b]), x8T(kx[b]), xvT_pack(vx[b])) for b in range(B)}
    in_maps = []
    for c in range(8):
        b, hg = divmod(c, 4)
        sl = slice(hg * OC, (hg + 1) * OC)
        in_maps.append({
            "xq8T": xb[b][0],
            "xk8T": xb[b][1],
            "xvT": xb[b][2],
            "w8q": w8_pack(Wq, sl),
            "w8k": w8_pack(Wk, sl),
            "wvp": np.ascontiguousarray(
                Wv[sl].T.astype(BF16).reshape(DCH, 128, OC)
                .transpose(1, 0, 2)),
            "bq2": np.ascontiguousarray(bq[sl].reshape(2, 128).T,
                                        dtype=np.float32),
            "bk2": np.ascontiguousarray(bk[sl].reshape(2, 128).T,
                                        dtype=np.float32),
            "bvr": np.ascontiguousarray(
                np.broadcast_to(bv[sl].astype(np.float32), (128, OC))),
            "mtiles": np.ascontiguousarray(mtiles.transpose(1, 0, 2)),
            "eye": eye,
        })

    trace = os.environ.get("BASS_KERNEL_TRACE") == "1"
    if trace:
        # warm run first: profiling start before the first executable load
        # wedges the load under axon
        run_bass_kernel_spmd(nc, in_maps, list(range(8)), trace=False)
    res = run_bass_kernel_spmd(nc, in_maps, list(range(8)), trace=trace)
    if trace:
        print(f"HW exec time: {res.exec_time_ns} ns "
              f"(mean {res.mean_exec_time_ns})")

    outp = np.zeros((B, S, D), np.float32)
    for c in range(8):
        b, hg = divmod(c, 4)
        o = np.asarray(res.results[c]["out"])      # [QC, 2, 128, 4, 128]
        o = o.transpose(0, 3, 2, 1, 4).reshape(S, OC)  # [qc,j,p] x [og,oc]
        outp[b, :, hg * OC:(hg + 1) * OC] = o.astype(np.float32)
    return outp
